# revision 20
# baseline (speedup 1.0000x reference)
"""TRN2 Bass kernel for nn_MultiHeadAttention_63977832841803 (sparse_attention).

Reference computation (H=8 heads, B=16, NQ=G=512, D=512, KD=VD=64, E=512):
  4 branches c: Q_c/K_c/V_c = per-head projections of q; s_c = (1/8) Q_c K_c^T;
  softmax over the concatenated (4*G) axis with per-branch masks (shared across
  heads); heads = sum_c attn_c V_c; out = sum_h heads_h @ W_out[h].

Sharding: pure data-parallel over batch B across 8 cores (2 batches/core), all
heads local, no collectives.

Per-core layout (tokens on the free axis):
  qT[d, n], Q^T/K^T[h*64+k, n], V packed per head as [v0:32 | ones | v32:64]
  so each head's AV runs as two column-tiled matmuls (M=33 incl. the
  denominator ones-row at PE cols 0:64, M=32 at cols 64:128) that execute
  concurrently on the PE; scores s^T[g, n] run as row-tiled head pairs
  (rows 0:64 / 64:128); exp on ScalarE only (scale=1/8 folded in); mask
  multiply on GpSimd (SBUF-only, keeps DVE free for PSUM drains); softmax
  denominators come out of the AV PSUM (row 32), reciprocal + a tiny K=2
  identity matmul broadcasts 1/den across partitions; final W_out contraction
  on PE with normalized heads^T stacked [h*64+v, n].

Host-side preprocessing (cheap numpy): batch shard, transpose q and the masks,
pack weights per-branch as [D, H*64], cast to bf16.
"""

import numpy as np
import ml_dtypes

import concourse.bass as bass  # noqa: F401  (engine types referenced via nc)
import concourse.mybir as mybir
from concourse import bacc, tile
from concourse.bass_utils import run_bass_kernel_spmd

BF16 = mybir.dt.bfloat16
F32 = mybir.dt.float32
AF = mybir.ActivationFunctionType

H, B, G, NQ = 8, 16, 512, 512
D, KD, VD, E = 512, 64, 64, 512
NORM = 1.0 / float(np.sqrt(KD))
NCORES = 8
BLOC = B // NCORES  # batches per core
NB = ml_dtypes.bfloat16

DC, GC, HP, NCH = 4, 4, 4, 4  # d-chunks, g-chunks, head-pairs, n-chunks


def build_kernel(reps=1):
    nc = bacc.Bacc()

    qt = nc.dram_tensor("qt", [BLOC, D, NQ], BF16, kind="ExternalInput")
    mt = nc.dram_tensor("mt", [4, BLOC, G, NQ], BF16, kind="ExternalInput")
    wq = nc.dram_tensor("wq", [4, D, H * KD], BF16, kind="ExternalInput")
    wk = nc.dram_tensor("wk", [4, D, H * KD], BF16, kind="ExternalInput")
    wv = nc.dram_tensor("wv", [4, D, H * VD], BF16, kind="ExternalInput")
    wo = nc.dram_tensor("wo", [H * VD, E], BF16, kind="ExternalInput")
    out = nc.dram_tensor("out", [BLOC, NQ, E], F32, kind="ExternalOutput")

    with tile.TileContext(nc) as tc:
        with (
            tc.tile_pool(name="wsb", bufs=1) as wsb,        # persistent weights
            tc.tile_pool(name="qsb", bufs=2) as qsb,        # qT tiles (4 tags)
            tc.tile_pool(name="msb", bufs=2) as msb,        # mask tiles (16 tags, prefetch)
            tc.tile_pool(name="proj", bufs=1) as projp,     # Q^T + K^T tiles (32 tags)
            tc.tile_pool(name="vaug", bufs=1) as vaugp,     # V tiles (16 tags)
            tc.tile_pool(name="expool", bufs=4) as expool,
            tc.tile_pool(name="small", bufs=1) as small,    # den/pbr per hp
            tc.tile_pool(name="headsp", bufs=1) as headsp,  # hn (4 tags)
            tc.tile_pool(name="osb", bufs=2) as osb,
            tc.tile_pool(name="pp", bufs=2, space="PSUM") as pp,    # proj/bcast/wout psum
            tc.tile_pool(name="ps", bufs=2, space="PSUM") as ps,    # score psum [128,1024]
            tc.tile_pool(name="pav", bufs=2, space="PSUM") as pav,  # AV accumulators
        ):
            # ---- persistent constants / weights ----
            qts_pre = []
            for dc in range(DC):
                t = qsb.tile([128, NQ], BF16, tag=f"qt{dc}", name=f"qtp{dc}")
                nc.sync.dma_start(t[0:64, :], qt[0, 128 * dc : 128 * dc + 64, :])
                nc.sync.dma_start(t[64:128, :], qt[0, 128 * dc + 64 : 128 * (dc + 1), :])
                qts_pre.append(t)
            wq_sb = [[wsb.tile([128, H * KD], BF16, tag=f"wq{c}{dc}", name=f"wq{c}{dc}") for dc in range(DC)] for c in range(4)]
            wk_sb = [[wsb.tile([128, H * KD], BF16, tag=f"wk{c}{dc}", name=f"wk{c}{dc}") for dc in range(DC)] for c in range(4)]
            wv_sb = [[wsb.tile([128, H * VD], BF16, tag=f"wv{c}{dc}", name=f"wv{c}{dc}") for dc in range(DC)] for c in range(4)]
            wo_sb = [wsb.tile([128, E], BF16, tag=f"wo{hc}", name=f"wo{hc}") for hc in range(4)]
            for c in range(4):
                for dc in range(DC):
                    sl = slice(128 * dc, 128 * (dc + 1))
                    nc.sync.dma_start(wq_sb[c][dc][:], wq[c, sl, :])
                    nc.sync.dma_start(wk_sb[c][dc][:], wk[c, sl, :])
                    nc.sync.dma_start(wv_sb[c][dc][:], wv[c, sl, :])
            for hc in range(4):
                nc.sync.dma_start(wo_sb[hc][:], wo[128 * hc : 128 * (hc + 1), :])
            # ones row for the 1/den partition broadcast (K=1 matmuls)
            ones1 = wsb.tile([1, 64], BF16, name="ones1")
            nc.vector.memset(ones1[:], 1.0)

            for bi, b in enumerate([bb for _ in range(reps) for bb in range(BLOC)]):
                # ---- load qT and masks for this batch ----
                if bi == 0:
                    qts = qts_pre
                else:
                    qts = []
                    for dc in range(DC):
                        t = qsb.tile([128, NQ], BF16, tag=f"qt{dc}")
                        nc.sync.dma_start(t[:], qt[b, 128 * dc : 128 * (dc + 1), :])
                        qts.append(t)
                mts = [[None] * GC for _ in range(4)]
                for c in range(4):
                    for gc in range(GC):
                        t = msb.tile([128, NQ], BF16, tag=f"m{c}{gc}")
                        nc.sync.dma_start(t[:], mt[c, b, 128 * gc : 128 * (gc + 1), :])
                        mts[c][gc] = t

                # ---- projections ----
                # V for all heads + Q/K for hp=0 up front; Q/K for hp+1 are
                # interleaved into hp's attention chunk loop so the PE's
                # exm-wait gaps are filled and ScalarE/DVE never starve.
                qT = [[None] * HP for _ in range(4)]
                kT = [[None] * HP for _ in range(4)]
                vab = [[None] * GC for _ in range(4)]

                def emit_qk_proj(hp):
                    hsl = slice(128 * hp, 128 * (hp + 1))
                    for c in range(4):
                        for which in range(2):
                            pqk = pp.tile([128, NQ], F32, tag="pproj")
                            w_sb = (wq_sb if which == 0 else wk_sb)[c]
                            for dc in range(DC):
                                nc.tensor.matmul(pqk[:], w_sb[dc][:, hsl], qts[dc][:],
                                                 start=(dc == 0), stop=(dc == DC - 1))
                            t = projp.tile([128, NQ], BF16,
                                           tag=(f"q{c}{hp}" if which == 0 else f"k{c}{hp}"))
                            nc.vector.tensor_copy(t[:], pqk[:])
                            if which == 0:
                                qT[c][hp] = t
                            else:
                                kT[c][hp] = t

                for c in range(4):
                    for gc in range(GC):
                        gsl = slice(128 * gc, 128 * (gc + 1))
                        pv = pp.tile([128, H * VD], F32, tag="pproj")
                        for dc in range(DC):
                            nc.tensor.matmul(pv[:], qts[dc][:, gsl], wv_sb[c][dc][:],
                                             start=(dc == 0), stop=(dc == DC - 1))
                        # per head: [v0:64 | ones] (65th col = denominator ones-row)
                        tv = vaugp.tile([128, H * 65], BF16, tag=f"v{c}{gc}")
                        tv3 = tv[:].rearrange("p (h v) -> p h v", v=65)
                        pv3 = pv[:].rearrange("p (h v) -> p h v", v=64)
                        nc.scalar.copy(tv3[:, :, 0:64], pv3[:, :, :])
                        nc.vector.memset(tv3[:, :, 64], 1.0)
                        vab[c][gc] = tv
                for hp in range(HP):
                    emit_qk_proj(hp)

                # ---- attention ----
                hns = [None] * HP
                chunks = [(c, gc) for c in range(4) for gc in range(GC)]
                NCHK = len(chunks)
                for hp in range(HP):
                    h0, h1 = 2 * hp, 2 * hp + 1
                    pa0 = pav.tile([65, NQ], F32, tag="av")
                    pa1 = pav.tile([65, NQ], F32, tag="av")
                    # software pipeline: emit chunk i+1's scores/exp/mask
                    # before chunk i's AV so the in-order PE never sits
                    # behind an AV matmul whose exm isn't ready yet.
                    exs = [None] * NCHK
                    for i in range(NCHK + 1):
                        if i < NCHK:
                            c, gc = chunks[i]
                            gsl = slice(128 * gc, 128 * (gc + 1))
                            sc = ps.tile([128, 2 * NQ], F32, tag="score")
                            nc.tensor.matmul(sc[:, 0:NQ], kT[c][hp][0:64, gsl],
                                             qT[c][hp][0:64, :], start=True, stop=True)
                            nc.tensor.matmul(sc[:, NQ : 2 * NQ], kT[c][hp][64:128, gsl],
                                             qT[c][hp][64:128, :], start=True, stop=True)
                            ex = expool.tile([128, 2 * NQ], BF16, tag="ex")
                            nc.scalar.activation(ex[:], sc[:], AF.Exp, scale=NORM)
                            # a slice of chunks goes to GpSimd (slower per-op
                            # but otherwise idle) to unload the DVE
                            eng = nc.gpsimd if (gc == 3 and c in (1, 3)) else nc.vector
                            if eng is nc.gpsimd:
                                ex3 = ex[:].rearrange("p (s n) -> p s n", s=2)
                                mbc = mts[c][gc][:, None, :].broadcast_to([128, 2, NQ])
                                eng.tensor_mul(ex3, ex3, mbc)
                            else:
                                eng.tensor_mul(ex[:, 0:NQ], ex[:, 0:NQ], mts[c][gc][:])
                                eng.tensor_mul(ex[:, NQ : 2 * NQ], ex[:, NQ : 2 * NQ],
                                               mts[c][gc][:])
                            exs[i] = ex
                        if i >= 1:
                            c, gc = chunks[i - 1]
                            ex = exs[i - 1]
                            st = i - 1 == 0
                            sp = i - 1 == NCHK - 1
                            nc.tensor.matmul(pa0[:], vab[c][gc][:, 65 * h0 : 65 * h0 + 65],
                                             ex[:, 0:NQ], start=st, stop=sp)
                            nc.tensor.matmul(pa1[:], vab[c][gc][:, 65 * h1 : 65 * h1 + 65],
                                             ex[:, NQ : 2 * NQ], start=st, stop=sp)
                    # ---- normalize: den rows -> partition broadcast -> 1/x ----
                    # pb lives in the score pool (a buffer is free at the hp
                    # boundary) so the interleaved proj chains keep pp to
                    # themselves.
                    den = small.tile([1, 2 * NQ], BF16, tag="den")
                    nc.vector.tensor_copy(den[0:1, 0:NQ], pa0[64:65, :])
                    nc.vector.tensor_copy(den[0:1, NQ : 2 * NQ], pa1[64:65, :])
                    pb = ps.tile([64, 2 * NQ], F32, tag="score")
                    nc.tensor.matmul(pb[:, 0:NQ], ones1[:], den[0:1, 0:NQ],
                                     start=True, stop=True)
                    nc.tensor.matmul(pb[:, NQ : 2 * NQ], ones1[:], den[0:1, NQ : 2 * NQ],
                                     start=True, stop=True)
                    pbr = small.tile([64, 2 * NQ], F32, tag="pbr")
                    nc.vector.reciprocal_approx_fast(pbr[:], pb[:])
                    # hn rows: [h0 v0:64 | h1 v0:64]
                    hn = headsp.tile([128, NQ], BF16, tag=f"hn{hp}")
                    nc.vector.tensor_mul(hn[0:64, :], pa0[0:64, :], pbr[:, 0:NQ])
                    nc.vector.tensor_mul(hn[64:128, :], pa1[0:64, :], pbr[:, NQ : 2 * NQ])
                    hns[hp] = hn

                # ---- final W_out contraction: out[n, e] ----
                for nch in range(NCH):
                    nsl = slice(128 * nch, 128 * (nch + 1))
                    pw = pp.tile([128, E], F32, tag="pproj")
                    for hc in range(4):
                        nc.tensor.matmul(pw[:], hns[hc][:, nsl], wo_sb[hc][:],
                                         start=(hc == 0), stop=(hc == 3))
                    ot = osb.tile([128, E], F32, tag="osb")
                    nc.vector.tensor_copy(ot[:], pw[:])
                    nc.sync.dma_start(out[b, nsl, :], ot[:])

    nc.finalize()
    return nc


_NC_CACHE = None


def _get_nc():
    global _NC_CACHE
    if _NC_CACHE is None:
        _NC_CACHE = build_kernel()
    return _NC_CACHE


def _prep_core_inputs(q, att, grp, spd, wq_np, wk_np, wv_np, wo_np, c0):
    """Host-side shard + transpose + pack for one core's batches [c0, c0+BLOC)."""
    sl = slice(c0, c0 + BLOC)
    qt = np.ascontiguousarray(q[sl].transpose(0, 2, 1)).astype(NB)  # [BLOC, D, NQ]
    # masks in [g, n] orientation per branch: m0=sparse^T, m1=att raw, m2=att^T, m3=group^T
    mt = np.empty((4, BLOC, G, NQ), dtype=NB)
    mt[0] = spd[sl].transpose(0, 2, 1)
    mt[1] = att[sl]
    mt[2] = att[sl].transpose(0, 2, 1)
    mt[3] = grp[sl].transpose(0, 2, 1)
    return {"qt": qt, "mt": mt, "wq": wq_np, "wk": wk_np, "wv": wv_np, "wo": wo_np}


def _pack_w(ws):
    # list of 4 (H, D, Kd) -> [4, D, H*Kd] bf16
    return np.stack([w.transpose(1, 0, 2).reshape(D, -1) for w in ws]).astype(NB)


def kernel(q, att_masks, group_masks, sparse_dist_masks,
           W_query, W_K, W_V, W_Q_ps, W_K_ps, W_V_ps,
           W_Q_sp, W_K_sp, W_V_sp, W_Q_pg, W_K_pg, W_V_pg, W_out,
           _want_results=False):
    q = np.asarray(q, dtype=np.float32)
    att = np.asarray(att_masks).astype(np.float32)
    grp = np.asarray(group_masks).astype(np.float32)
    spd = np.asarray(sparse_dist_masks).astype(np.float32)

    wq_np = _pack_w([np.asarray(w, np.float32) for w in (W_query, W_Q_ps, W_Q_sp, W_Q_pg)])
    wk_np = _pack_w([np.asarray(w, np.float32) for w in (W_K, W_K_ps, W_K_sp, W_K_pg)])
    wv_np = _pack_w([np.asarray(w, np.float32) for w in (W_V, W_V_ps, W_V_sp, W_V_pg)])
    wo_np = np.asarray(W_out, np.float32).reshape(H * VD, E).astype(NB)

    in_maps = [
        _prep_core_inputs(q, att, grp, spd, wq_np, wk_np, wv_np, wo_np, BLOC * i)
        for i in range(NCORES)
    ]
    nc = _get_nc()
    res = run_bass_kernel_spmd(nc, in_maps, list(range(NCORES)))
    out = np.concatenate([res.results[i]["out"] for i in range(NCORES)], axis=0)
    if _want_results:
        return out, res
    return out
